# revision 27
# baseline (speedup 1.0000x reference)
"""Causal self-attention kernel for 8 Trainium2 NeuronCores.

Problem: B=4, T=2048, C=1024, H=16 heads (D=64).
Sharding: data-parallel over batch (4) x tensor-parallel over heads (2 groups
of 8 heads). Core c handles batch c//2, head-group c%2. Each core computes
qkv for its 8 heads, full causal attention on TxT scores, and its partial
projection output; the host sums the two head-group partials per batch.

Pipeline design (v2): one fused stream. The 16 attention strips
(pair, i-chunk) run ic-major; qkv emission, per-strip softmax
normalization, and the output projection are all drip-fed between
attention tiles through a deadline-ordered pending queue so the scalar
engine (exp) and PE never idle and the PE HAM clock stays warm.

Per-core layout:
  - activations feature-major: xT [C, T]; qT/kT pair-packed
    [128=(2 heads x 64d), pair, T]; v as [T, jt, head, 65] (65th col = ones,
    so the PV matmul also produces the softmax denominator l).
  - scores are computed transposed (S.T = [j, i]); the two heads of a pair
    run concurrently in the PE via row-group tiling (contraction 64 each).
  - causal masking: exp first, then multiply the diagonal 128x128 block by
    a 0/1 bf16 mask (cheap DVE op off the fp32 path).
  - softmax normalization: l rows are staged per strip and batched per level;
    1/l = exp(-ln l) on ACT (one natural_log_exp table set), DMA-packed to
    partition 0, fp16 PE ones-broadcast, in-place multiply of yU. The final
    (pair 3, level 3) uses a DMA-free chain at partition 96 to shorten the
    tail before the last projection.
  - all matmuls bf16 with fp32 PSUM accumulation; output stored bf16.
"""

import sys

if "/opt/trn_rl_repo" not in sys.path:
    sys.path.insert(0, "/opt/trn_rl_repo")

from contextlib import ExitStack

import ml_dtypes
import numpy as np

import concourse.bass as bass
import concourse.mybir as mybir
import concourse.tile as tile
from concourse.bass_utils import run_bass_kernel_spmd

BF16 = mybir.dt.bfloat16
F16 = mybir.dt.float16
F32 = mybir.dt.float32
F32R = mybir.dt.float32r
NP_BF16 = ml_dtypes.bfloat16

P = 128
B, T, C = 4, 2048, 1024
H = 16
D = 64
HL = 8            # heads per core
NPAIR = HL // 2   # head pairs per core
NL = HL * D       # 512: local qkv width
CT = C // P       # 8 contraction tiles over C
DT = NL // P      # 4 contraction tiles over local head dims
NTO = C // P      # 8 output tiles for proj
TCH = T // 512    # 4 t-chunks
NJT = T // P      # 16 j tiles
LOOKAHEAD = 10    # strips a pending unit may run early


def _split_excess_waits(nc, limit=1):
    """This walrus build supports a single sem-wait per instruction; move
    excess waits emitted by Tile onto preceding same-engine NoOps."""
    n = 0
    for bb in nc.main_func.blocks:
        out = []
        changed = False
        for inst in bb.instructions:
            si = inst.sync_info
            if si is not None and len(si.on_wait) > limit:
                waits = list(si.on_wait)
                excess, keep = waits[:-limit], waits[-limit:]
                for i in range(0, len(excess), limit):
                    out.append(
                        mybir.InstNoOp(
                            name=f"waitsplit_{n}",
                            ins=[],
                            outs=[],
                            engine=inst.engine,
                            sync_info=mybir.SyncInfo(
                                on_wait=excess[i : i + limit], on_update=[]
                            ),
                        )
                    )
                    n += 1
                si.on_wait = keep
                changed = True
            out.append(inst)
        if changed:
            bb.instructions = out
    return n


def build_nc(split_waits=True):
    nc = bass.Bass()
    AF = mybir.ActivationFunctionType
    ADD = mybir.AluOpType.add
    MULT = mybir.AluOpType.mult

    xT = nc.dram_tensor("xT", [P, TCH, CT, 512], BF16, kind="ExternalInput")
    # wq/wk are pair-major so each pair's block is one contiguous DMA
    wq = nc.dram_tensor("wq", [P, NPAIR, CT, P], BF16, kind="ExternalInput")
    wk = nc.dram_tensor("wk", [P, NPAIR, CT, P], BF16, kind="ExternalInput")
    wv = nc.dram_tensor("wv", [P, CT, NL], BF16, kind="ExternalInput")
    wp = nc.dram_tensor("wp", [P, DT, C], BF16, kind="ExternalInput")
    bq = nc.dram_tensor("bq", [P, NPAIR], F32, kind="ExternalInput")
    bk = nc.dram_tensor("bk", [P, NPAIR], F32, kind="ExternalInput")
    bv = nc.dram_tensor("bv", [P, NL], F32, kind="ExternalInput")
    bp = nc.dram_tensor("bp", [P, NTO], F32, kind="ExternalInput")
    mskb = nc.dram_tensor("mskb", [P, P], BF16, kind="ExternalInput")
    outT = nc.dram_tensor("outT", [P, NTO, T], BF16, kind="ExternalOutput")

    with tile.TileContext(nc) as tc, ExitStack() as ctx:
        persist = ctx.enter_context(tc.tile_pool(name="persist", bufs=1))
        # PSUM budget (8 banks of [128, 2KB]):
        #   spsum tag "s"  [P, 2, 512] f32 x2 bufs = 4 banks (scores)
        #   ypsum yA/yB    [D+1, 512] f32 x1 buf  = 2 banks (PV accum)
        #   wpsum tag "w"  [P, 512]   f32 x2 bufs = 2 banks (qkv/proj/bcast)
        spsum = ctx.enter_context(tc.tile_pool(name="spsum", bufs=2, space="PSUM"))
        ypsum = ctx.enter_context(tc.tile_pool(name="ypsum", bufs=1, space="PSUM"))
        wpsum = ctx.enter_context(tc.tile_pool(name="wpsum", bufs=2, space="PSUM"))
        work = ctx.enter_context(tc.tile_pool(name="work", bufs=3))

        # ---- persistent SBUF tensors ----
        qT = persist.tile([P, NPAIR, T], BF16)   # [2x64d, pair, t]
        kT = persist.tile([P, NPAIR, T], BF16)
        vA = persist.tile([P, NJT, HL, P], BF16)  # [j, jt, head, d|0|ones@96]
        yU = persist.tile([P, DT, T], BF16)  # y.T pair-packed; normalized in place
        onesP = persist.tile([P, D], F16)    # lhsT rows for PE partition-broadcast
        xs = persist.tile([P, TCH, CT, 512], BF16)
        wqs = persist.tile([P, NPAIR, CT, P], BF16)
        wks = persist.tile([P, NPAIR, CT, P], BF16)
        wvs = persist.tile([P, CT, NL], BF16)
        wps = persist.tile([P, DT, C], BF16)
        bqs = persist.tile([P, NPAIR], F32)
        bks = persist.tile([P, NPAIR], F32)
        bvs = persist.tile([P, NL], F32)
        bps = persist.tile([P, NTO], F32)
        msks = persist.tile([P, 1, P], BF16)
        wrm = persist.tile([1, 8], F32)
        wrmo = persist.tile([1, 8], BF16)

        # ---- DMA issue (sync: x + small consts + outputs; scalar: weights)
        for h in range(4):
            nc.scalar.dma_start(
                wvs[:, 2 * h : 2 * h + 2, :], wv[:, 2 * h : 2 * h + 2, :]
            )
            nc.sync.dma_start(
                xs[:, 0, 2 * h : 2 * h + 2, :], xT[:, 0, 2 * h : 2 * h + 2, :]
            )
        nc.scalar.dma_start(wqs[:, 0], wq[:, 0])
        nc.sync.dma_start(bvs[:], bv[:])
        nc.sync.dma_start(bqs[:], bq[:])
        nc.sync.dma_start(bks[:], bk[:])
        nc.sync.dma_start(bps[:], bp[:])
        nc.sync.dma_start(msks[:, 0, :], mskb[:])
        nc.scalar.dma_start(wks[:, 0], wk[:, 0])
        for pr in range(1, NPAIR):
            nc.scalar.dma_start(wks[:, pr], wk[:, pr])
            nc.scalar.dma_start(wqs[:, pr], wq[:, pr])
        for tc_i in range(1, TCH):
            nc.sync.dma_start(xs[:, tc_i, 0:4, :], xT[:, tc_i, 0:4, :])
            nc.sync.dma_start(xs[:, tc_i, 4:8, :], xT[:, tc_i, 4:8, :])
        nc.scalar.dma_start(wps[:], wp[:])

        # exp/ln table preload + constants while DMAs land
        nc.vector.memset(wrm[:], 1.0)
        nc.scalar.activation(wrmo[:], wrm[:], AF.Ln)
        nc.scalar.activation(wrmo[:], wrm[:], AF.Exp, scale=0.125)
        nc.vector.memset(vA[:, :, :, D:P], 0.0)
        nc.vector.memset(vA[:, :, :, 96:97], 1.0)
        nc.vector.memset(onesP[:], 1.0)

        def xsl(ct, t0, n):  # slice of xs covering [t0, t0+n) at c-tile ct
            tc_i, o = divmod(t0, 512)
            return xs[:, tc_i, ct, o : o + n]

        # ---- work units -------------------------------------------------
        def emit_v(tt):
            ps = wpsum.tile([P, 512], F32, tag="w", name=f"vps{tt}")
            for ct in range(CT):
                nc.tensor.matmul(
                    ps[:],
                    lhsT=xsl(ct, tt * P, P),
                    rhs=wvs[:, ct, :],
                    start=(ct == 0),
                    stop=(ct == CT - 1),
                )
            nc.vector.tensor_tensor(
                out=vA[:, tt, :, 0:D],
                in0=ps.rearrange("p (h d) -> p h d", h=HL),
                in1=bvs.rearrange("p (h d) -> p h d", h=HL),
                op=ADD,
            )

        qk_done = set()

        def emit_q(pr, tc_i):
            qk_done.add(("q", pr, tc_i))
            ps = wpsum.tile([P, 512], F32, tag="w", name=f"qps{pr}_{tc_i}")
            for ct in range(CT):
                nc.tensor.matmul(
                    ps[:],
                    lhsT=wqs[:, pr, ct, :],
                    rhs=xs[:, tc_i, ct, :],
                    start=(ct == 0),
                    stop=(ct == CT - 1),
                )
            nc.vector.tensor_scalar(
                out=qT[:, pr, tc_i * 512 : (tc_i + 1) * 512],
                in0=ps[:],
                scalar1=bqs[:, pr : pr + 1],
                scalar2=None,
                op0=ADD,
            )

        def emit_k(pr, tc_i):
            qk_done.add(("k", pr, tc_i))
            ps = wpsum.tile([P, 512], F32, tag="w", name=f"kps{pr}_{tc_i}")
            for ct in range(CT):
                nc.tensor.matmul(
                    ps[:],
                    lhsT=wks[:, pr, ct, :],
                    rhs=xs[:, tc_i, ct, :],
                    start=(ct == 0),
                    stop=(ct == CT - 1),
                )
            nc.vector.tensor_scalar(
                out=kT[:, pr, tc_i * 512 : (tc_i + 1) * 512],
                in0=ps[:],
                scalar1=bks[:, pr : pr + 1],
                scalar2=None,
                op0=ADD,
            )

        lrow_tiles = {}
        lvr_tiles = {}
        lst33 = []

        def emit_normA(ic):
            # softmax denominators 1/l = exp(-ln l) on ACT over the level's
            # gathered l rows (all 4 pairs for ic<3; pairs 0-2 at ic=3), then
            # DMA-pack to partition 0 for the fp16 PE ones-broadcast.
            lrow = lrow_tiles.pop(ic)
            nr = 6 if ic == TCH - 1 else HL
            lnl = work.tile([HL, 512], F32, tag="lnl", name=f"lnl{ic}", bufs=2)
            nc.scalar.activation(lnl[0:nr, :], lrow[0:nr, :], AF.Ln)
            linv = work.tile([HL, 512], F16, tag="linv", name=f"linv{ic}", bufs=2)
            nc.scalar.activation(linv[0:nr, :], lnl[0:nr, :], AF.Exp, scale=-1.0)
            lvr = work.tile([1, HL, 512], F16, tag="lvr", name=f"lvr{ic}", bufs=2)
            lvr_tiles[ic] = lvr
            nc.sync.dma_start(lvr[0:1, 0:nr, :], linv[0:nr, :])

        def emit_normA2():
            # tail pair (3,3): DMA-free chain — Ln/Exp in place at partition
            # 64 on the staged l rows, fp16 broadcast with a partition-64 ones
            # row (32-aligned, so auto tile_position is legal).
            lst = lst33[0]
            lnd = work.tile([97, 2, 512], F32, tag="lnd", name="lnd33", bufs=1)
            nc.scalar.activation(lnd[96:97, :, :], lst[96:97, :, :], AF.Ln)
            lvh = work.tile([97, 2, 512], F16, tag="lvh", name="lvh33", bufs=1)
            lst33.append(lvh)
            nc.scalar.activation(
                lvh[96:97, :, :], lnd[96:97, :, :], AF.Exp, scale=-1.0
            )

        def emit_normB(ic, pr):
            i_sl = slice(ic * 512, (ic + 1) * 512)
            tailp = ic == TCH - 1 and pr == NPAIR - 1
            if tailp:
                lvh = lst33[1]
                rA, rB = lvh[96:97, 0, :], lvh[96:97, 1, :]
                ones = onesP[96:97, :]
            else:
                lvr = lvr_tiles[ic]
                rA, rB = lvr[0:1, 2 * pr, :], lvr[0:1, 2 * pr + 1, :]
                ones = onesP[0:1, :]
            tp = (96, 0) if tailp else None
            lbA = wpsum.tile([D, 512], F32, tag="w", name=f"lbA{pr}_{ic}")
            nc.tensor.matmul(
                lbA[:], lhsT=ones, rhs=rA, start=True, stop=True,
                tile_position=tp,
            )
            lbB = wpsum.tile([D, 512], F32, tag="w", name=f"lbB{pr}_{ic}")
            nc.tensor.matmul(
                lbB[:], lhsT=ones, rhs=rB, start=True, stop=True,
                tile_position=tp,
            )
            nc.vector.tensor_tensor(
                out=yU[0:D, pr, i_sl], in0=yU[0:D, pr, i_sl],
                in1=lbA[:], op=MULT,
            )
            nc.vector.tensor_tensor(
                out=yU[D:P, pr, i_sl], in0=yU[D:P, pr, i_sl],
                in1=lbB[:], op=MULT,
            )

        def emit_proj(ic, nt):
            ps = wpsum.tile([P, 512], F32, tag="w", name=f"pps{ic}_{nt}")
            i_sl = slice(ic * 512, (ic + 1) * 512)
            for dt in range(DT):
                nc.tensor.matmul(
                    ps[:],
                    lhsT=wps[:, dt, nt * P : (nt + 1) * P],
                    rhs=yU[:, dt, i_sl],
                    start=(dt == 0),
                    stop=(dt == DT - 1),
                )
            ot = work.tile([P, 512], BF16, tag="out", name=f"ot{ic}_{nt}")
            nc.vector.tensor_scalar(
                out=ot[:],
                in0=ps[:],
                scalar1=bps[:, nt : nt + 1],
                scalar2=None,
                op0=ADD,
            )
            # tail level: spread output DMAs over both queues (ACT is idle)
            eng = nc.scalar if (ic == TCH - 1 and nt % 2) else nc.sync
            eng.dma_start(outT[:, nt, i_sl], ot[:])

        # pending units: (deadline, ready, seq, fn); strips are numbered
        # s = 4*ic + pr (ic-major). A unit may be emitted once sidx >= ready
        # and must be emitted before strip `deadline` runs.
        pending = []
        seq = 0

        def add_unit(deadline, ready, fn):
            nonlocal seq
            pending.append([deadline, ready, seq, fn])
            seq += 1

        for tt in range(4, 16):
            add_unit(4 * (tt // 4), 0, (lambda t: lambda: emit_v(t))(tt))
        for tc_i in range(TCH):
            for pr in range(NPAIR):
                if (pr, tc_i) == (0, 0):
                    continue
                s = 4 * tc_i + pr
                add_unit(s, 0, (lambda p, t: lambda: emit_q(p, t))(pr, tc_i))
                add_unit(s, 0, (lambda p, t: lambda: emit_k(p, t))(pr, tc_i))
        for ic in range(TCH):
            last = ic == TCH - 1
            ra = 4 * ic + (3 if last else 4)
            add_unit(4 * ic + (4 if last else 5), ra, (lambda i: lambda: emit_normA(i))(ic))
            if last:
                add_unit(4 * ic + 5, 4 * ic + 4, emit_normA2)
            for pr in range(NPAIR):
                rb = 4 * ic + (4 if (last and pr == NPAIR - 1) else (3 if last else 5))
                add_unit(
                    4 * ic + (5 if last else 6), rb,
                    (lambda i, p: lambda: emit_normB(i, p))(ic, pr),
                )
            for nt in range(NTO):
                add_unit(
                    4 * ic + 7,
                    4 * ic + 6,
                    (lambda i, n: lambda: emit_proj(i, n))(ic, nt),
                )
        pending.sort(key=lambda u: (u[0], u[1], u[2]))

        def drain_due(sidx):
            rest = []
            for u in pending:
                if u[0] <= sidx:
                    u[3]()
                else:
                    rest.append(u)
            pending[:] = rest

        def drip_one(sidx):
            for idx, u in enumerate(pending):
                if u[1] <= sidx:
                    if u[0] <= sidx + LOOKAHEAD:
                        pending.pop(idx)
                        u[3]()
                    return

        # ---- preamble: v tiles 0-3 and q/k for (pair 0, chunk 0) ----
        emit_v(0)
        emit_v(1)
        emit_q(0, 0)
        emit_v(2)
        emit_v(3)
        emit_k(0, 0)

        # ---- attention strips, ic-major; scores pipeline continuously
        # across strip boundaries (next strip's first tiles are emitted in
        # this strip's empty prefetch slots) ----
        order = [(4 * ic + pr, pr, ic) for ic in range(TCH) for pr in range(NPAIR)]
        score_tiles = {}

        def emit_score_pair(si, jt):
            _, pr, ic = order[si]
            i0 = ic * 512
            st = spsum.tile([P, 2, 512], F32, tag="s", name=f"st{si}_{jt}")
            score_tiles[(si, jt)] = st
            ow = max(0, jt * P - i0)
            j_sl = slice(jt * P, (jt + 1) * P)
            i_sl = slice(i0 + ow, i0 + 512)
            nc.tensor.matmul(
                st[:, 0, ow:512],
                lhsT=kT[0:D, pr, j_sl],
                rhs=qT[0:D, pr, i_sl],
                start=True, stop=True,
                tile_position=(0, 0),
            )
            nc.tensor.matmul(
                st[:, 1, ow:512],
                lhsT=kT[D:P, pr, j_sl],
                rhs=qT[D:P, pr, i_sl],
                start=True, stop=True,
                tile_position=(64, 0),
            )

        def strip(pr, ic, sidx):
            hA, hB = 2 * pr, 2 * pr + 1
            njt = 4 * ic + 4
            i0 = ic * 512
            yA = ypsum.tile([P, 512], F32, tag="yA", name=f"yA{sidx}")
            yB = ypsum.tile([P, 512], F32, tag="yB", name=f"yB{sidx}")

            for jt in range(min(2, njt)):
                if (sidx, jt) not in score_tiles:
                    emit_score_pair(sidx, jt)
            # two j-tiles per iteration: 4 score matmuls then 4 PV matmuls,
            # halving the PE's 64-row/128-row tile-mode flips (better FWL)
            pts = {}
            for j2 in range(0, njt, 2):
                for jt in (j2, j2 + 1):
                    st = score_tiles.pop((sidx, jt))
                    ow = max(0, jt * P - i0)
                    pt = work.tile(
                        [P, 2, 512], BF16, tag="p", name=f"pt{sidx}_{jt}"
                    )
                    pts[jt] = pt
                    nc.scalar.activation(
                        pt[:, :, ow:512], st[:, :, ow:512], AF.Exp, scale=0.125
                    )
                    if jt >= 4 * ic:  # diagonal tile: zero p above diagonal
                        nc.vector.tensor_tensor(
                            out=pt[:, :, ow : ow + P],
                            in0=pt[:, :, ow : ow + P],
                            in1=msks[:].to_broadcast([P, 2, P]),
                            op=MULT,
                        )
                drip_one(sidx)
                if j2 == 0:
                    drip_one(sidx)
                for jt in (j2, j2 + 1):
                    if jt + 2 < njt:
                        emit_score_pair(sidx, jt + 2)
                for jt in (j2, j2 + 1):
                    pt = pts.pop(jt)
                    ow = max(0, jt * P - i0)
                    nc.tensor.matmul(
                        yA[:, ow:512],
                        lhsT=vA[:, jt, hA, :],
                        rhs=pt[:, 0, ow:512],
                        start=(jt == 0),
                        stop=(jt == njt - 1),
                    )
                    nc.tensor.matmul(
                        yB[:, ow:512],
                        lhsT=vA[:, jt, hB, :],
                        rhs=pt[:, 1, ow:512],
                        start=(jt == 0),
                        stop=(jt == njt - 1),
                    )

            # strip tail: stash unnormalized y; stage l rows (psum partition
            # 64) then DMA-spread them into the level's [8, 512] gather tile.
            if pr == 0:
                lrow_tiles[ic] = work.tile(
                    [HL, 512], F32, tag="lrow", name=f"lrow{ic}", bufs=2
                )
            i_sl = slice(i0, i0 + 512)
            lst = work.tile([97, 2, 512], F32, tag="lst", name=f"lst{sidx}")
            nc.vector.tensor_copy(lst[96:97, 0, :], yA[96:97, :])
            if ic == TCH - 1 and pr == NPAIR - 1:
                # tail: second l copy on the (now idle) ACT engine so the
                # Ln/Exp chain starts ~0.7us earlier and the PE gap stays
                # under the HAM re-throttle window
                nc.scalar.activation(lst[96:97, 1, :], yB[96:97, :], AF.Identity)
                lst33.append(lst)  # consumed by the DMA-free tail chain
            else:
                nc.vector.tensor_copy(lst[96:97, 1, :], yB[96:97, :])
                nc.sync.dma_start(
                    lrow_tiles[ic][2 * pr : 2 * pr + 2, :],
                    lst[96:97, :, :],
                )
            nc.vector.tensor_copy(yU[0:D, pr, i_sl], yA[0:D, :])
            nc.vector.tensor_copy(yU[D:P, pr, i_sl], yB[0:D, :])

        for ic in range(TCH):
            for pr in range(NPAIR):
                sidx = 4 * ic + pr
                drain_due(sidx)
                strip(pr, ic, sidx)
                # bridge the strip boundary with independent PE work so the
                # PSUM WAR wait can't idle the PE (HAM would re-throttle)
                drip_one(sidx)
                drip_one(sidx)

        # tail: remaining norm + proj units in deadline order
        for u in pending:
            u[3]()
        pending.clear()

    if split_waits:
        _split_excess_waits(nc, 1)
    return nc


def shard_inputs(x, w_attn, b_attn, w_proj, b_proj):
    """Build the 8 per-core input dicts (core = 2*batch + head_group)."""
    x = np.asarray(x, dtype=np.float32)
    w_attn = np.asarray(w_attn, dtype=np.float32)
    b_attn = np.asarray(b_attn, dtype=np.float32)
    w_proj = np.asarray(w_proj, dtype=np.float32)
    b_proj = np.asarray(b_proj, dtype=np.float32)

    # multiplicative causal mask for the diagonal 128x128 block of S.T
    # ([j, i]): 1 where j <= i, 0 above the diagonal.
    pp = np.arange(P)
    mskb = (pp[:, None] <= pp[None, :]).astype(NP_BF16)

    def wtile(w2d, ncols):  # [C_rows, ncols] -> [P, rows//P, ncols] bf16
        r = w2d.shape[0]
        return np.ascontiguousarray(
            w2d.reshape(r // P, P, ncols).transpose(1, 0, 2)
        ).astype(NP_BF16)

    in_maps = []
    for core in range(8):
        b, hg = divmod(core, 2)
        q0 = hg * NL
        xt = np.ascontiguousarray(x[b].T)  # [C, T]
        wq_t = wtile(w_attn[:, q0 : q0 + NL], NL)  # [P, CT, NL]
        wk_t = wtile(w_attn[:, C + q0 : C + q0 + NL], NL)
        # pair-major: [P, NPAIR, CT, P]
        wq_p = np.ascontiguousarray(
            wq_t.reshape(P, CT, NPAIR, P).transpose(0, 2, 1, 3)
        )
        wk_p = np.ascontiguousarray(
            wk_t.reshape(P, CT, NPAIR, P).transpose(0, 2, 1, 3)
        )
        m = {
            "xT": np.ascontiguousarray(
                xt.reshape(CT, P, TCH, 512).transpose(1, 2, 0, 3)
            ).astype(NP_BF16),
            "wq": wq_p,
            "wk": wk_p,
            "wv": wtile(w_attn[:, 2 * C + q0 : 2 * C + q0 + NL], NL),
            "wp": wtile(w_proj[q0 : q0 + NL, :], C),
            "bq": np.ascontiguousarray(
                b_attn[q0 : q0 + NL].reshape(NPAIR, P).T
            ).astype(np.float32),
            "bk": np.ascontiguousarray(
                b_attn[C + q0 : C + q0 + NL].reshape(NPAIR, P).T
            ).astype(np.float32),
            "bv": np.broadcast_to(
                b_attn[2 * C + q0 : 2 * C + q0 + NL], (P, NL)
            ).astype(np.float32),
            "bp": (
                np.ascontiguousarray(b_proj.reshape(NTO, P).T).astype(np.float32)
                if hg == 0
                else np.zeros((P, NTO), np.float32)
            ),
            "mskb": mskb,
        }
        in_maps.append(m)
    return in_maps


def unshard_output(results):
    """Combine 8 per-core outT [P, NTO, T] bf16 partials into [B, T, C]."""
    out = np.empty((B, T, C), dtype=np.float32)
    for b in range(B):
        acc = results[2 * b]["outT"].astype(np.float32) + results[
            2 * b + 1
        ]["outT"].astype(np.float32)
        # [P, NTO, T] -> [C, T] -> [T, C]
        out[b] = acc.transpose(1, 0, 2).reshape(C, T).T
    return out


_NC_CACHE = {}


def kernel(x, w_attn, b_attn, w_proj, b_proj):
    if "nc" not in _NC_CACHE:
        _NC_CACHE["nc"] = build_nc()
    nc = _NC_CACHE["nc"]
    in_maps = shard_inputs(x, w_attn, b_attn, w_proj, b_proj)
    res = run_bass_kernel_spmd(nc, in_maps, core_ids=list(range(8)))
    return unshard_output(res.results)
